# revision 45
# baseline (speedup 1.0000x reference)
"""Causal single-head attention (B=4, T=4096, D=1024) on 8 trn2 NeuronCores.

Sharding: 2 cores per batch element, split by key-block PARITY (flash-style):
  core = 2*b + p ; p in {0,1}
  Each core computes, for ALL 4096 queries of batch b, the partial
  (unnormalized) attention output over its 16 key blocks {128*(2u+p)} and the
  partial softmax row-sums. Host merges: O = (O_0 + O_1) / (rs_0 + rs_1).

v10 design (on top of v3's M-fold + bf16 + parity split):
  Key-side M-fold: scores = q.k^T = x Wq^T Wk x̃^T = x (x̃ M^T)^T with
    M = Wq^T Wk precomputed on host; no q projection at all.
  Single interleaved PE stream: chunks ascend j = 0..15 with one projection
    group (vproj(j), plus kproj before chunks 4/8/12) emitted just-in-time
    before every attn(j) — each chunk boundary gets PE filler so the
    drain-copy / exp serial chains never expose the tensor engine. Any PE
    stall also costs ~3us of mid-pstate throughput, so micro-gaps matter.
  Score blocks run in pairs sharing ONE PSUM accumulation group per ss
    bank (a group start marks the whole 2KB bank pending-zero; per-block
    starts would chain each block's start on the previous block's exp).
  Diagonal block at position 4 so its mask-add -> exp chain has slack.
  Last chunk processed as two interleaved 128-query halves (4 quarter
    regions of the ss bank, both halves' scores emitted before their exps):
    each half covers the other's latencies and only a 0.5 MB drain is
    exposed at the end.
  O emitted bf16 (host merge upcasts): halves output DMA; each drain is a
    single flat 256 KB descriptor (row-contiguous [128, 1024] bf16).
  All bulk inputs pre-packed host-side into single-plane DMA layouts
    (descriptor issue is ~0.6us per 2-D plane, serialized per queue); the
    whole input stream rides the sync queue in consumption order, which
    doubles as a bandwidth prioritizer. Rowsum ones-columns via memset.
  fp8 was evaluated and is numerically dead here: any single e4m3 operand
    (scores / P / V) alone exceeds the 2e-2 tolerance (measured 2-3.7e-2).
"""

import sys

sys.path.insert(0, "/opt/trn_rl_repo")

import numpy as np
import ml_dtypes
from contextlib import ExitStack

import concourse.tile as tile
from concourse import bacc, mybir
from concourse.bass_utils import run_bass_kernel_spmd

P = 128
D = 1024
T = 4096
B = 4
NDB = D // P  # 8 feature blocks
NCB = D // P  # 8 contraction blocks
NKB = 16  # key blocks per core (parity half of 32)
QC = 256  # query-chunk columns
NQC = T // QC  # 16
F32 = mybir.dt.float32
BF16 = mybir.dt.bfloat16
EXPSCALE = 1.0 / 32.0  # 1/sqrt(D)
EXP = mybir.ActivationFunctionType.Exp

_CACHED_NC = None
_LAST_RES = None


def _build_program():
    nc = bacc.Bacc("TRN2", target_bir_lowering=False, debug=False, num_devices=8)

    # All bulk inputs pre-packed host-side so every DMA is a single 2-D
    # descriptor plane (descriptor issue costs ~0.6us per plane, serialized
    # per queue).
    xq_d = nc.dram_tensor("XQ", [NQC, P, NCB, QC], BF16, kind="ExternalInput").ap()
    kt_d = nc.dram_tensor("KT", [4, P, NCB, 512], BF16, kind="ExternalInput").ap()
    m_d = nc.dram_tensor("MT2", [NDB, P, NCB, P], BF16, kind="ExternalInput").ap()
    wv_d = nc.dram_tensor("WV", [2, P, NCB, 512], BF16, kind="ExternalInput").ap()
    mask_d = nc.dram_tensor("mask", [P, QC], F32, kind="ExternalInput").ap()
    o_d = nc.dram_tensor("O", [T, D], BF16, kind="ExternalOutput").ap()
    rs_d = nc.dram_tensor("rs", [T, 1], F32, kind="ExternalOutput").ap()

    xq_r = xq_d.rearrange("j p a c -> p j a c")  # [128, 16, 8, 256]
    kt_r = kt_d.rearrange("g p a c -> p g a c")  # [128, 4, 8, 512]
    m_r = m_d.rearrange("g p a c -> p g a c")  # [128, ig 8, cb 8, 128]
    wv_r = wv_d.rearrange("v p a c -> p v a c")  # [128, 2, 8, 512]

    with tile.TileContext(nc) as tc, ExitStack() as ctx:
        kv = ctx.enter_context(tc.tile_pool(name="kv", bufs=1))
        xp = ctx.enter_context(tc.tile_pool(name="xp", bufs=5))
        wp = ctx.enter_context(tc.tile_pool(name="wp", bufs=2))
        pp = ctx.enter_context(tc.tile_pool(name="pp", bufs=6))
        stg = ctx.enter_context(tc.tile_pool(name="stg", bufs=4))
        psum = ctx.enter_context(tc.tile_pool(name="psum", bufs=1, space="PSUM"))

        mask_t = kv.tile([P, QC], F32, tag="mask")
        mT_t = kv.tile([P, NDB, NCB, P], BF16, tag="mT")  # M^T ig-major, 16 KiB
        kt_t = kv.tile([P, 4, NCB, 512], BF16, tag="kt")  # x̃^T g-slabs, 32 KiB
        kpT_t = kv.tile([P, NCB, T // 2], BF16, tag="kpT")  # k'^T, 32 KiB
        v_t = kv.tile([P, NKB, D + 4], BF16, tag="vt")  # 32.1 KiB

        # ---- startup DMAs: ALL on the sync queue, in consumption order. The
        # ~0.6us serialized issue per descriptor doubles as a bandwidth
        # prioritizer: the transfers gating the first kproj matmuls get the
        # HBM to themselves before the bulk slabs start competing. ----
        xqs = {}

        def fetch(j):
            if j <= 15 and j not in xqs:
                t = xp.tile([P, NCB, QC], BF16, tag="x", name=f"xq{j}")
                nc.sync.dma_start(t[:], xq_r[:, j])
                xqs[j] = t

        nc.sync.dma_start(mT_t[:, 0], m_r[:, 0])
        nc.sync.dma_start(kt_t[:, 0, 0:2], kt_r[:, 0, 0:2])
        nc.sync.dma_start(kt_t[:, 0, 2:4], kt_r[:, 0, 2:4])
        nc.sync.dma_start(mT_t[:, 1], m_r[:, 1])
        nc.sync.dma_start(kt_t[:, 0, 4:6], kt_r[:, 0, 4:6])
        nc.sync.dma_start(kt_t[:, 0, 6:8], kt_r[:, 0, 6:8])
        # rowsum ones-columns via strided memsets (no DMA descriptors)
        nc.vector.memset(v_t[:, :, D : D + 1], 1.0)
        nc.vector.memset(v_t[:, :, D + 1 : D + 4], 0.0)
        for ig in range(2, NDB):
            nc.sync.dma_start(mT_t[:, ig], m_r[:, ig])
        wvs = []
        for vc in range(2):
            wv = wp.tile([P, NCB, 512], BF16, tag="wv", name=f"wv{vc}")
            nc.sync.dma_start(wv[:], wv_r[:, vc])
            wvs.append(wv)
        nc.scalar.dma_start(mask_t[:], mask_d[:])
        fetch(0)
        fetch(1)
        fetch(2)
        fetch(3)

        prot = [0]  # kproj/vproj PSUM double-buffer rotation

        def kproj(g):
            for ig in range(NDB):
                ps = psum.tile([P, 512], F32, tag=f"p{prot[0] % 2}", name=f"kps{g}_{ig}")
                prot[0] += 1
                for cb in range(NCB):
                    nc.tensor.matmul(
                        ps[:],
                        mT_t[:, ig, cb, :],
                        kt_t[:, g, cb, :],
                        start=(cb == 0),
                        stop=(cb == NCB - 1),
                    )
                nc.vector.tensor_copy(kpT_t[:, ig, g * 512 : (g + 1) * 512], ps[:])

        def vproj(kb):
            for vc in range(2):
                ps = psum.tile([P, 512], F32, tag=f"p{prot[0] % 2}", name=f"vps{vc}_{kb}")
                prot[0] += 1
                for cb in range(NCB):
                    nc.tensor.matmul(
                        ps[:],
                        kt_t[:, kb // 4, cb, (kb % 4) * P : (kb % 4 + 1) * P],
                        wvs[vc][:, cb, :],
                        start=(cb == 0),
                        stop=(cb == NCB - 1),
                    )
                nc.vector.tensor_copy(v_t[:, kb, vc * 512 : (vc + 1) * 512], ps[:])

        def attn(j, qoff, qn, btag0, rr=None):
            """Score+AV for chunk j over xq columns [qoff, qoff+qn).

            btag0: first of 2*(qn//128) consecutive acc bank tags (b0..b3).
            rr: optional existing row-sum bank tile (shared by the two
            half-chunks of j=15). Returns (acc, qoff, nsub, rr)."""
            fetch(j + 4)
            xq = xqs[j]
            if j != 15 or qoff == P:
                xqs.pop(j)
            nsub = qn // P
            roff = btag0 // 2
            if rr is None:
                rr = psum.tile([P, 8], F32, tag="rr", name=f"rr_{j}_{qoff}")
            ss = psum.tile([P, 512], F32, tag="ss", name=f"ss_{j}_{qoff}")
            acc = {}
            for sub in range(nsub):
                acc[sub, 0] = psum.tile(
                    [P, 512], F32, tag=f"b{btag0 + 2 * sub}", name=f"a0_{j}_{qoff}_{sub}"
                )
                acc[sub, 1] = psum.tile(
                    [P, 512], F32, tag=f"b{btag0 + 2 * sub + 1}", name=f"a1_{j}_{qoff}_{sub}"
                )
                acc[sub, 2] = rr[:, 4 * (roff + sub) : 4 * (roff + sub) + 4]

            def av(u, pt_t, first, last):
                for sub in range(nsub):
                    lhs = pt_t[:, sub * P : (sub + 1) * P]
                    nc.tensor.matmul(
                        acc[sub, 0][:], lhs, v_t[:, u, 0:512],
                        start=first, stop=last, skip_group_check=True,
                    )
                    nc.tensor.matmul(
                        acc[sub, 1][:], lhs, v_t[:, u, 512:1024],
                        start=first, stop=last, skip_group_check=True,
                    )
                    # rowsum groups share one PSUM bank; start marks the WHOLE
                    # 2KB bank pending-zero, so only sub0 may issue it — sub1's
                    # first write lazily zeroes its own region off that mark.
                    nc.tensor.matmul(
                        acc[sub, 2], lhs, v_t[:, u, D : D + 4],
                        start=first and sub == 0, stop=last, skip_group_check=True,
                    )

            # Diagonal block at position 4: its mask-add -> exp serial chain
            # (queued on Vector behind the previous chunk's drain copies) needs
            # ~4 score blocks of slack before the next score group reuses its
            # PSUM bank, else the PE stalls ~0.3-0.6us at every chunk boundary.
            dpos = min(j, 4)
            uorder = list(range(dpos)) + [j] + list(range(dpos, j))
            # Score blocks run in PAIRS sharing ONE accumulation group across
            # the whole ss bank: a group start marks the entire 2KB bank
            # pending-zero, so per-block starts would chain each block's start
            # on the previous block's exp read (exposed ~0.6us at chunk start
            # where no av work covers it). With one start per pair, the second
            # block's first matmul lazily zeroes its own half off the pair's
            # start mark.
            pts = {}
            for i, u in enumerate(uorder):
                st = ss[:, (i % 2) * 256 : (i % 2) * 256 + qn]
                pair_first = i % 2 == 0
                pair_last = (i % 2 == 1) or (i == len(uorder) - 1)
                for db in range(NDB):
                    nc.tensor.matmul(
                        st,
                        kpT_t[:, db, u * P : (u + 1) * P],
                        xq[:, db, qoff : qoff + qn],
                        start=(db == 0 and pair_first),
                        stop=(db == NDB - 1 and pair_last),
                        skip_group_check=True,
                    )
                if u == j:
                    nc.vector.tensor_add(st, st, mask_t[:, qoff : qoff + qn])
                pt = pp.tile([P, qn], BF16, tag="pt", name=f"pt{j}_{qoff}_{u}")
                nc.scalar.activation(pt[:], st, EXP, scale=EXPSCALE)
                pts[u] = pt
                if i >= 2:
                    av(uorder[i - 2], pts.pop(uorder[i - 2]),
                       first=(i == 2), last=False)
            n = len(uorder)
            if n >= 2:
                av(uorder[n - 2], pts.pop(uorder[n - 2]), first=(n == 2), last=False)
            av(uorder[n - 1], pts.pop(uorder[n - 1]), first=(n == 1), last=True)
            return acc, qoff, nsub, rr

        def drain(acc, qoff, nsub, j, final=False):
            # All drain copies on Vector (gpsimd cannot access PSUM): the
            # Scalar queue stays clear so exp(i0) fires immediately at chunk
            # start — the score pipeline serializes on it via PSUM bank reuse.
            # The final drain splits copies Vector/Scalar for latency.
            dma = nc.sync.dma_start if final else nc.gpsimd.dma_start
            for sub in range(nsub):
                row = j * QC + qoff + sub * P
                rt = stg.tile([P, 1], F32, tag="rt", name=f"rt{j}_{qoff}_{sub}")
                copy_eng = nc.scalar.copy if final else nc.vector.tensor_copy
                copy_eng(rt[:], acc[sub, 2][:, 0:1])  # first: frees the rr bank
                big = stg.tile([P, D], BF16, tag="stage", name=f"ot_{j}_{qoff}_{sub}")
                nc.vector.tensor_copy(big[:, 0:512], acc[sub, 0][:])
                if final:
                    nc.scalar.copy(big[:, 512:1024], acc[sub, 1][:])
                else:
                    nc.vector.tensor_copy(big[:, 512:1024], acc[sub, 1][:])
                dma(o_d[row : row + P, :], big[:])
                dma(rs_d[row : row + P, :], rt[:])

        def attn15():
            """Chunk 15 as two interleaved 128-query halves: each half's score
            blocks provide the exp/av latency cover the other half needs, and
            half A's drain hides behind half B's av tail, leaving only a 0.5MB
            final drain exposed. ss splits into four [P,128] quarter-regions
            (two per half)."""
            xq = xqs.pop(15)
            rr = psum.tile([P, 8], F32, tag="rr", name="rr_15")
            ss = psum.tile([P, 512], F32, tag="ss", name="ss_15")
            acc = {}
            for h in range(2):
                acc[h] = {
                    (0, 0): psum.tile([P, 512], F32, tag=f"b{2 * h}", name=f"a0_15_{h}"),
                    (0, 1): psum.tile([P, 512], F32, tag=f"b{2 * h + 1}", name=f"a1_15_{h}"),
                    (0, 2): rr[:, 4 * h : 4 * h + 4],
                }

            def av(h, u, pt_t, first, last):
                a = acc[h]
                nc.tensor.matmul(
                    a[0, 0][:], pt_t[:], v_t[:, u, 0:512],
                    start=first, stop=last, skip_group_check=True,
                )
                nc.tensor.matmul(
                    a[0, 1][:], pt_t[:], v_t[:, u, 512:1024],
                    start=first, stop=last, skip_group_check=True,
                )
                # only half A starts the shared rr bank (bank-wide mark)
                nc.tensor.matmul(
                    a[0, 2], pt_t[:], v_t[:, u, D : D + 4],
                    start=first and h == 0, stop=last, skip_group_check=True,
                )

            uorder = [0, 1, 2, 3, 15] + list(range(4, 15))
            n = len(uorder)
            pts = {}
            for i, u in enumerate(uorder):
                # both halves' score groups FIRST, then both exps: a pair-start
                # marks the whole shared bank, so emitting B's start after A's
                # exp would serialize it on that exp.
                sts = {}
                for h in range(2):
                    qoff = h * P
                    r = h * 2 + i % 2
                    st = sts[h] = ss[:, r * P : (r + 1) * P]
                    for db in range(NDB):
                        nc.tensor.matmul(
                            st,
                            kpT_t[:, db, u * P : (u + 1) * P],
                            xq[:, db, qoff : qoff + P],
                            start=(db == 0 and i % 2 == 0),
                            stop=(db == NDB - 1 and (i % 2 == 1 or i == n - 1)),
                            skip_group_check=True,
                        )
                for h in range(2):
                    qoff = h * P
                    if u == 15:
                        nc.vector.tensor_add(sts[h], sts[h], mask_t[:, qoff : qoff + P])
                    pt = pp.tile([P, P], BF16, tag="pt", name=f"pt15_{h}_{u}")
                    nc.scalar.activation(pt[:], sts[h], EXP, scale=EXPSCALE)
                    pts[h, u] = pt
                if i >= 2:
                    up = uorder[i - 2]
                    for h in range(2):
                        av(h, up, pts.pop((h, up)), first=(i == 2), last=False)
            for k in (n - 2, n - 1):
                av(0, uorder[k], pts.pop((0, uorder[k])), first=False, last=(k == n - 1))
            drain(acc[0], 0, 1, 15)
            for k in (n - 2, n - 1):
                av(1, uorder[k], pts.pop((1, uorder[k])), first=False, last=(k == n - 1))
            drain(acc[1], P, 1, 15, final=True)

        # ---- interleaved PE stream: every chunk boundary gets a projection
        # filler (just-in-time vproj(j) right before attn(j)) so the exposed
        # exp->score-start chain at chunk starts always hides behind PE work.
        kproj(0)
        vproj(0)
        for j in range(15):
            acc, qoff, nsub, rr = attn(j, 0, QC, 0)
            drain(acc, qoff, nsub, j)
            if j <= 2:
                # late key slabs: issued on the scalar ring BEHIND chunk j's
                # exps, so their transfers start only after the critical
                # early input stream has drained (in-order queue as timer)
                nc.scalar.dma_start(kt_t[:, j + 1], kt_r[:, j + 1])
            if j + 1 in (4, 8, 12):
                kproj((j + 1) // 4)
            vproj(j + 1)
        attn15()

    nc.finalize()
    return nc


def _get_program():
    global _CACHED_NC
    if _CACHED_NC is None:
        _CACHED_NC = _build_program()
    return _CACHED_NC


def _masks():
    neg = np.float32(-1e30)
    tri = np.where(np.triu(np.ones((P, P), dtype=bool)), np.float32(0), neg)
    keep = np.zeros((P, P), dtype=np.float32)
    drop = np.full((P, P), neg, dtype=np.float32)
    return (
        np.ascontiguousarray(np.concatenate([tri, keep], axis=1)),  # even core
        np.ascontiguousarray(np.concatenate([drop, tri], axis=1)),  # odd core
    )


def kernel(x, Wq, Wk, Wv):
    out, _ = _run(x, Wq, Wk, Wv, trace=False)
    return out


def _run(x, Wq, Wk, Wv, trace=False, keep_res=False):
    BF = ml_dtypes.bfloat16
    x = np.asarray(x, dtype=np.float32)
    M = (np.asarray(Wq, np.float64).T @ np.asarray(Wk, np.float64)).astype(np.float32)
    A = np.ascontiguousarray(M.T.astype(BF))  # [j, i]
    # [ig, p(=j in cb), cb, c(=i in ig)]
    MT2 = np.ascontiguousarray(
        A.reshape(NCB, P, NDB, P).transpose(2, 1, 0, 3)
    )
    WvT_bf = np.asarray(Wv, np.float32).T.astype(BF)
    m_even, m_odd = _masks()

    WV2 = np.ascontiguousarray(WvT_bf.reshape(NCB, P, 2, 512).transpose(2, 1, 0, 3))
    nc = _get_program()
    in_maps = []
    for core in range(8):
        b, p = core // 2, core % 2
        xT = x[b].T.astype(BF)  # [D, T]
        xTk = xT.reshape(D, T // P, P)[:, p::2, :].reshape(D, T // 2)
        # [j, p, a(feature blk), c] / [g, p, cb, c] plane-packed
        XQ = np.ascontiguousarray(xT.reshape(NCB, P, NQC, QC).transpose(2, 1, 0, 3))
        KT = np.ascontiguousarray(xTk.reshape(NCB, P, 4, 512).transpose(2, 1, 0, 3))
        in_maps.append(
            {
                "XQ": XQ,
                "KT": KT,
                "MT2": MT2,
                "WV": WV2,
                "mask": m_even if p == 0 else m_odd,
            }
        )

    res = run_bass_kernel_spmd(nc, in_maps, core_ids=list(range(8)), trace=trace)
    if keep_res:
        global _LAST_RES
        _LAST_RES = res
    out = np.empty((B, T, D), dtype=np.float32)
    for b in range(B):
        O0 = np.asarray(res.results[2 * b]["O"], dtype=np.float32)
        O1 = np.asarray(res.results[2 * b + 1]["O"], dtype=np.float32)
        rs0, rs1 = res.results[2 * b]["rs"], res.results[2 * b + 1]["rs"]
        out[b] = (O0 + O1) / (rs0 + rs1)
    return out, res.exec_time_ns
